# revision 4
# baseline (speedup 1.0000x reference)
"""Trainium2 Bass kernel for nn_BranchingLayer (gnn_message_passing).

Reference computation (shapes hardcoded from the spec):
  x:[786432,32] f32, global_features:[2048,16], parents_idxs:[524288] i32,
  W1:[48,128], b1:[128], W2:[128,128], b2:[128]
  parents = x[parents_idxs]                      # [524288, 32], row i = (p, b) = (i//2048, i%2048)
  h  = leaky_relu(concat(parents, g[b]) @ W1 + b1, 0.01)
  proj = h @ W2 + b2 + repeat_interleave(parents, 4, -1)
  children[(p*4+br)*2048 + b, f] = proj[p*2048+b, br*32+f]
  out = concat([x, children], 0)                 # [2883584, 32]

Strategy: shard the 256 parents over 8 cores (32 parents/core).  Each
core's x-slice and output-slice are contiguous.  Host pre-transposes x
to feature-major [33, 2048] tiles per parent (row 32 = ones, folding the
biases into the matmuls).  Device chain per parent/quarter(512 cols):
  psum1 = W1x^T.xt + W1g^T.gt          (K=33 / K=16 accumulate, b1 via ones row)
  h1    = relu(psum1)                  (ACT engine)
  psum2 = (0.99 W2)^T.h1 + G2^T.gt + ER^T.xt
          where leaky(z) = 0.99*relu(z) + 0.01*z is folded:
          G2 = 0.01*W1g@W2, ER = E_res + 0.01*W1x@W2 (+ bias row), exactly.
  bt    = DVE 32x32 block-transpose(psum2)   -> layout matches row-major DRAM
  DMA per branch: bt[32br:32br+32, :] -> children[(p*4+br)*2048 : +2048, :32]
"""

import numpy as np

BATCH = 2048
NPAR = 256
NF = 32
NG = 16
NBR = 4
OFF = 262144
NCORES = 8
PPC = NPAR // NCORES          # parents per core
QW = 512                      # matmul free-dim (quarter of batch)
NQ = BATCH // QW

_CACHE = {}


def _build_nc(ppc=PPC, reps=1):
    import concourse.bacc as bacc
    import concourse.bass as bass
    import concourse.mybir as mybir
    import concourse.tile as tile
    from contextlib import ExitStack, nullcontext

    dt = mybir.dt.float32
    nc = bacc.Bacc("TRN2", target_bir_lowering=False, debug=False)

    xt_d = nc.dram_tensor("xt", [ppc, 33, BATCH], dt, kind="ExternalInput")
    gt_d = nc.dram_tensor("gt", [NG, BATCH], dt, kind="ExternalInput")
    w1_d = nc.dram_tensor("w1", [33, 128], dt, kind="ExternalInput")
    w1g_d = nc.dram_tensor("w1g", [NG, 128], dt, kind="ExternalInput")
    w2_d = nc.dram_tensor("w2", [128, 128], dt, kind="ExternalInput")
    er_d = nc.dram_tensor("er", [33, 128], dt, kind="ExternalInput")
    g2_d = nc.dram_tensor("g2", [NG, 128], dt, kind="ExternalInput")
    out_d = nc.dram_tensor("out", [ppc * NBR * BATCH, NF], dt, kind="ExternalOutput")

    with tile.TileContext(nc) as tc, ExitStack() as ctx:
        wpool = ctx.enter_context(tc.tile_pool(name="w", bufs=1))
        xpool = ctx.enter_context(tc.tile_pool(name="x", bufs=3))
        hpool = ctx.enter_context(tc.tile_pool(name="h", bufs=4))
        btpool = ctx.enter_context(tc.tile_pool(name="bt", bufs=2))
        p1pool = ctx.enter_context(
            tc.tile_pool(name="p1", bufs=2, space=bass.MemorySpace.PSUM)
        )
        p2pool = ctx.enter_context(
            tc.tile_pool(name="p2", bufs=2, space=bass.MemorySpace.PSUM)
        )

        w1_t = wpool.tile([33, 128], dt, tag="w1")
        nc.sync.dma_start(w1_t[:], w1_d[:])
        w1g_t = wpool.tile([NG, 128], dt, tag="w1g")
        nc.sync.dma_start(w1g_t[:], w1g_d[:])
        w2_t = wpool.tile([128, 128], dt, tag="w2")
        nc.sync.dma_start(w2_t[:], w2_d[:])
        er_t = wpool.tile([33, 128], dt, tag="er")
        nc.sync.dma_start(er_t[:], er_d[:])
        g2_t = wpool.tile([NG, 128], dt, tag="g2")
        nc.sync.dma_start(g2_t[:], g2_d[:])
        gt_t = wpool.tile([NG, BATCH], dt, tag="gt")
        nc.sync.dma_start(gt_t[:], gt_d[:])

        rep_ctx = tc.For_i(0, reps, 1) if reps > 1 else nullcontext()
        with rep_ctx:
            _parent_loop_body(nc, tc, ppc, dt, mybir, xpool, hpool, btpool,
                              p1pool, p2pool, xt_d, out_d,
                              w1_t, w1g_t, w2_t, er_t, g2_t, gt_t)
    nc.compile()
    return nc


def _parent_loop_body(nc, tc, ppc, dt, mybir, xpool, hpool, btpool,
                      p1pool, p2pool, xt_d, out_d,
                      w1_t, w1g_t, w2_t, er_t, g2_t, gt_t):
    import concourse.bass as bass  # noqa

    if True:
        for p in range(ppc):
            xt_t = xpool.tile([33, BATCH], dt, tag="xt")
            nc.sync.dma_start(xt_t[:], xt_d[p])
            bt_t = btpool.tile([128, BATCH], dt, tag="bt")
            for q in range(NQ):
                s = slice(q * QW, (q + 1) * QW)
                ps1 = p1pool.tile([128, QW], dt, tag="ps1")
                nc.tensor.matmul(ps1[:], w1_t[:], xt_t[:, s], start=True, stop=False)
                nc.tensor.matmul(ps1[:], w1g_t[:], gt_t[:, s], start=False, stop=True)
                h1 = hpool.tile([128, QW], dt, tag="h1")
                nc.scalar.activation(
                    h1[:], ps1[:], mybir.ActivationFunctionType.Relu
                )
                ps2 = p2pool.tile([128, QW], dt, tag="ps2")
                nc.tensor.matmul(ps2[:], w2_t[:], h1[:], start=True, stop=False)
                nc.tensor.matmul(ps2[:], g2_t[:], gt_t[:, s], start=False, stop=False)
                nc.tensor.matmul(ps2[:], er_t[:], xt_t[:, s], start=False, stop=True)
                nc.vector.transpose(bt_t[:, s], ps2[:])
            for br in range(NBR):
                row0 = (p * NBR + br) * BATCH
                dst = out_d[row0 : row0 + BATCH, :].rearrange(
                    "(m bm) f -> bm m f", bm=32
                )
                src = bt_t[32 * br : 32 * (br + 1), :].rearrange(
                    "bm (m f) -> bm m f", f=32
                )
                nc.sync.dma_start(dst, src)


def _get_nc():
    if "nc" not in _CACHE:
        _CACHE["nc"] = _build_nc()
    return _CACHE["nc"]


def _pack_inputs(x, global_features, parents_idxs, W1, b1, W2, b2, ppc=PPC):
    """Build the per-core input maps (host-side sharding + layout)."""
    x = np.asarray(x, np.float32)
    g = np.asarray(global_features, np.float32)
    idx = np.asarray(parents_idxs)
    W1 = np.asarray(W1, np.float32)
    b1 = np.asarray(b1, np.float32)
    W2 = np.asarray(W2, np.float32)
    b2 = np.asarray(b2, np.float32)

    n_rows = NPAR * BATCH
    exp = np.arange(n_rows, dtype=np.int64)
    if np.array_equal(idx, exp + OFF):
        parents = x[OFF : OFF + n_rows]
    else:
        parents = x[idx]  # general gather
    gi = idx.astype(np.int64) % BATCH
    if not np.array_equal(gi, np.tile(np.arange(BATCH, dtype=np.int64), NPAR)):
        return None, parents, gi  # signal: generic fallback needed

    # xt: [NPAR, 33, BATCH]; rows 0-31 feature-major x, row 32 ones
    xt = np.empty((NPAR, 33, BATCH), np.float32)
    xt[:, :32] = parents.reshape(NPAR, BATCH, NF).transpose(0, 2, 1)
    xt[:, 32] = 1.0
    gt = np.ascontiguousarray(g.T)

    W1f = W1.astype(np.float64)
    W2f = W2.astype(np.float64)
    lin = 0.01 * (W1f @ W2f)  # [48, 128]
    w1 = np.concatenate([W1[:32], b1[None]], axis=0).astype(np.float32)
    w1g = np.ascontiguousarray(W1[32:48])
    w2 = (0.99 * W2f).astype(np.float32)
    er = np.zeros((33, 128), np.float64)
    jj = np.arange(128)
    er[jj // 4, jj] = 1.0
    er[:32] += lin[:32]
    er[32] = b2.astype(np.float64) + 0.01 * (b1.astype(np.float64) @ W2f)
    er = er.astype(np.float32)
    g2 = lin[32:48].astype(np.float32)

    ncores = NPAR // ppc
    in_maps = []
    for c in range(ncores):
        in_maps.append(
            {
                "xt": xt[c * ppc : (c + 1) * ppc],
                "gt": gt,
                "w1": w1,
                "w1g": w1g,
                "w2": w2,
                "er": er,
                "g2": g2,
            }
        )
    return in_maps, parents, gi


def _numpy_fallback(x, global_features, parents_idxs, W1, b1, W2, b2):
    x = np.asarray(x, np.float32)
    g = np.asarray(global_features, np.float32)
    idx = np.asarray(parents_idxs).astype(np.int64)
    pf = x[idx]
    pg = g[idx % BATCH]
    h = np.concatenate([pf, pg], axis=-1) @ np.asarray(W1, np.float32) + b1
    h = np.where(h > 0, h, 0.01 * h).astype(np.float32)
    proj = h @ np.asarray(W2, np.float32) + b2
    proj = proj + np.repeat(pf, NBR, axis=-1)
    m = proj.reshape(NPAR, BATCH, NF * NBR)
    m = np.swapaxes(m, 1, 2)
    m = m.reshape(NPAR * NBR, NF, BATCH)
    m = np.swapaxes(m, 1, 2)
    children = m.reshape(NPAR * NBR * BATCH, NF)
    return np.concatenate([x, children], axis=0).astype(np.float32)


def kernel(x, global_features, parents_idxs, W1, b1, W2, b2):
    in_maps, _, _ = _pack_inputs(x, global_features, parents_idxs, W1, b1, W2, b2)
    if in_maps is None:
        return _numpy_fallback(x, global_features, parents_idxs, W1, b1, W2, b2)

    from concourse.bass_utils import run_bass_kernel_spmd

    nc = _get_nc()
    res = run_bass_kernel_spmd(nc, in_maps, core_ids=list(range(NCORES)))
    _CACHE["last_result"] = res

    x = np.asarray(x, np.float32)
    out = np.empty((x.shape[0] + NPAR * NBR * BATCH, NF), np.float32)
    out[: x.shape[0]] = x
    base = x.shape[0]
    per = PPC * NBR * BATCH
    for c in range(NCORES):
        out[base + c * per : base + (c + 1) * per] = res.results[c]["out"]
    return out


# revision 11
# speedup vs baseline: 4.0073x; 4.0073x over previous
"""Trainium2 Bass kernel for nn_BranchingLayer (gnn_message_passing).

Reference computation (shapes hardcoded from the spec):
  x:[786432,32] f32, global_features:[2048,16], parents_idxs:[524288] i32,
  W1:[48,128], b1:[128], W2:[128,128], b2:[128]
  parents = x[parents_idxs]                # [524288, 32], row i = (p, b)
  h  = leaky_relu(concat(parents, g[b]) @ W1 + b1, 0.01)
  proj = h @ W2 + b2 + repeat_interleave(parents, 4, -1)
  children[(p*4+br)*2048 + b, f] = proj[p*2048+b, br*32+f]
  out = concat([x, children], 0)           # [2883584, 32]

Design (v2):
 * Shard the 256 parents over 8 cores (32/core); per-core x and output
   slices are contiguous.
 * bf16 matmuls (fp32 PE runs at 1/4 rate), fp32 PSUM accumulation.
   leaky(z) = 0.99*relu(z) + 0.01*z with the linear term folded into the
   residual matmul weights (host-precomputed in f64).  The residual
   (out += x) is kept fp32-exact by a hi/lo bf16 split of x.
 * Feature-major compute: per parent/quarter, psum1[128f,512] =
   W1'^T.xt, h1 = relu(psum1) (bf16), psum2 = W2'^T.h1 + ER^T.xt +
   E^T.xlo; DVE 32x32 block-transpose psum2 -> bt.
 * Batch columns are host-permuted: column 32c+d holds row 64d+c.  After
   the 32x32 block transpose, partition 32*br+d holds rows 64d..64d+64
   of branch br contiguously -> the output DMA is 32 descriptors x 8KB
   per branch (full line rate), one DMA per (parent, branch).
"""

import numpy as np

BATCH = 2048
NPAR = 256
NF = 32
NG = 16
NBR = 4
OFF = 262144
NCORES = 8
PPC = NPAR // NCORES          # parents per core
QW = 512                      # matmul free-dim (quarter of batch)
NQ = BATCH // QW
XROWS = 81                    # packed DRAM rows: 0-31 x_hi, 32-47 g_hi, 48 ones, 49-80 x_lo
XTILE = 96                    # SBUF tile rows: x_lo lives at 64-95 (matmul base-partition rule)

_CACHE = {}


def _build_nc(ppc=PPC, reps=1):
    import concourse.bacc as bacc
    import concourse.bass as bass
    import concourse.mybir as mybir
    import concourse.tile as tile
    from contextlib import ExitStack, nullcontext

    bf = mybir.dt.float16
    f32 = mybir.dt.float32
    nc = bacc.Bacc("TRN2", target_bir_lowering=False, debug=False)

    xt_d = nc.dram_tensor("xt", [ppc, XROWS, BATCH], bf, kind="ExternalInput")
    w1_d = nc.dram_tensor("w1", [49, 128], bf, kind="ExternalInput")
    w2_d = nc.dram_tensor("w2", [128, 128], bf, kind="ExternalInput")
    er_d = nc.dram_tensor("er", [49, 128], bf, kind="ExternalInput")
    el_d = nc.dram_tensor("el", [32, 128], bf, kind="ExternalInput")
    out_d = nc.dram_tensor("out", [ppc * NBR * BATCH, NF], f32, kind="ExternalOutput")

    with tile.TileContext(nc) as tc, ExitStack() as ctx:
        wpool = ctx.enter_context(tc.tile_pool(name="w", bufs=1))
        xpool = ctx.enter_context(tc.tile_pool(name="x", bufs=3))
        hpool = ctx.enter_context(tc.tile_pool(name="h", bufs=4))
        btpool = ctx.enter_context(tc.tile_pool(name="bt", bufs=3))
        p1pool = ctx.enter_context(
            tc.tile_pool(name="p1", bufs=2, space=bass.MemorySpace.PSUM)
        )
        p2pool = ctx.enter_context(
            tc.tile_pool(name="p2", bufs=2, space=bass.MemorySpace.PSUM)
        )

        w1_t = wpool.tile([49, 128], bf, tag="w1")
        nc.sync.dma_start(w1_t[:], w1_d[:])
        w2_t = wpool.tile([128, 128], bf, tag="w2")
        nc.sync.dma_start(w2_t[:], w2_d[:])
        er_t = wpool.tile([49, 128], bf, tag="er")
        nc.sync.dma_start(er_t[:], er_d[:])
        el_t = wpool.tile([96, 128], bf, tag="el")
        nc.sync.dma_start(el_t[64:96, :], el_d[:])

        rep_ctx = tc.For_i(0, reps, 1) if reps > 1 else nullcontext()
        with rep_ctx:
            for p in range(ppc):
                xt_t = xpool.tile([XTILE, BATCH], bf, tag="xt")
                nc.sync.dma_start(xt_t[:49, :], xt_d[p, :49])
                nc.sync.dma_start(xt_t[64:96, :], xt_d[p, 49:81])
                bt_t = btpool.tile([128, BATCH], f32, tag="bt")
                for q in range(NQ):
                    s = slice(q * QW, (q + 1) * QW)
                    ps1 = p1pool.tile([128, QW], f32, tag="ps1")
                    nc.tensor.matmul(
                        ps1[:], w1_t[:], xt_t[:49, s], start=True, stop=True
                    )
                    h1 = hpool.tile([128, QW], bf, tag="h1")
                    nc.scalar.activation(
                        h1[:], ps1[:], mybir.ActivationFunctionType.Relu
                    )
                    ps2 = p2pool.tile([128, QW], f32, tag="ps2")
                    nc.tensor.matmul(ps2[:], w2_t[:], h1[:], start=True, stop=False)
                    nc.tensor.matmul(
                        ps2[:], er_t[:], xt_t[:49, s], start=False, stop=False
                    )
                    nc.tensor.matmul(
                        ps2[:], el_t[64:96, :], xt_t[64:96, s], start=False, stop=True
                    )
                    nc.vector.transpose(bt_t[:, s], ps2[:])
                for br in range(NBR):
                    row0 = (p * NBR + br) * BATCH
                    dst = out_d[row0 : row0 + BATCH, :].rearrange(
                        "(d c) f -> d (c f)", d=32
                    )
                    src = bt_t[32 * br : 32 * (br + 1), :]
                    nc.sync.dma_start(dst, src)
    nc.compile()
    return nc


def _get_nc():
    if "nc" not in _CACHE:
        _CACHE["nc"] = _build_nc()
    return _CACHE["nc"]


def _perm_cols(a):
    """Permute the trailing batch axis: position 32c+d <- row 64d+c."""
    shp = a.shape[:-1]
    return np.ascontiguousarray(
        a.reshape(*shp, 32, 64).swapaxes(-1, -2).reshape(*shp, BATCH)
    )


def _pack_inputs(x, global_features, parents_idxs, W1, b1, W2, b2, ppc=PPC):
    """Build the per-core input maps (host-side sharding + layout)."""
    bf16 = np.float16
    x = np.asarray(x, np.float32)
    g = np.asarray(global_features, np.float32)
    idx = np.asarray(parents_idxs)
    W1 = np.asarray(W1, np.float32)
    b1 = np.asarray(b1, np.float32)
    W2 = np.asarray(W2, np.float32)
    b2 = np.asarray(b2, np.float32)

    n_rows = NPAR * BATCH
    exp = np.arange(n_rows, dtype=np.int64)
    if np.array_equal(idx, exp + OFF):
        parents = x[OFF : OFF + n_rows]
    else:
        parents = x[idx]  # general gather
    gi = idx.astype(np.int64) % BATCH
    if not np.array_equal(gi, np.tile(np.arange(BATCH, dtype=np.int64), NPAR)):
        return None

    # Feature-major per-parent x with permuted batch columns
    xf = parents.reshape(NPAR, BATCH, NF).transpose(0, 2, 1)  # [P, 32, B]
    xf = _perm_cols(xf)
    x_hi = xf.astype(bf16)
    x_lo = (xf - x_hi.astype(np.float32)).astype(bf16)
    g_hi = _perm_cols(np.ascontiguousarray(g.T)).astype(bf16)  # [16, B]

    xt = np.empty((NPAR, XROWS, BATCH), bf16)
    xt[:, :32] = x_hi
    xt[:, 32:48] = g_hi[None]
    xt[:, 48] = np.float32(1.0)
    xt[:, 49:81] = x_lo

    W1f = W1.astype(np.float64)
    W2f = W2.astype(np.float64)
    lin = 0.01 * (W1f @ W2f)  # [48, 128]
    w1 = np.concatenate([W1, b1[None]], axis=0).astype(bf16)  # [49, 128]
    w2 = (0.99 * W2f).astype(bf16)
    er = np.zeros((49, 128), np.float64)
    jj = np.arange(128)
    er[jj // 4, jj] = 1.0
    er[:48] += lin
    er[48] = b2.astype(np.float64) + 0.01 * (b1.astype(np.float64) @ W2f)
    er = er.astype(bf16)
    el = np.zeros((32, 128), bf16)
    el[jj // 4, jj] = np.float32(1.0)

    ncores = NPAR // ppc
    in_maps = []
    for c in range(ncores):
        in_maps.append(
            {
                "xt": xt[c * ppc : (c + 1) * ppc],
                "w1": w1,
                "w2": w2,
                "er": er,
                "el": el,
            }
        )
    return in_maps


def _numpy_fallback(x, global_features, parents_idxs, W1, b1, W2, b2):
    x = np.asarray(x, np.float32)
    g = np.asarray(global_features, np.float32)
    idx = np.asarray(parents_idxs).astype(np.int64)
    pf = x[idx]
    pg = g[idx % BATCH]
    h = np.concatenate([pf, pg], axis=-1) @ np.asarray(W1, np.float32) + b1
    h = np.where(h > 0, h, 0.01 * h).astype(np.float32)
    proj = h @ np.asarray(W2, np.float32) + b2
    proj = proj + np.repeat(pf, NBR, axis=-1)
    m = proj.reshape(NPAR, BATCH, NF * NBR)
    m = np.swapaxes(m, 1, 2)
    m = m.reshape(NPAR * NBR, NF, BATCH)
    m = np.swapaxes(m, 1, 2)
    children = m.reshape(NPAR * NBR * BATCH, NF)
    return np.concatenate([x, children], axis=0).astype(np.float32)


def kernel(x, global_features, parents_idxs, W1, b1, W2, b2):
    in_maps = _pack_inputs(x, global_features, parents_idxs, W1, b1, W2, b2)
    if in_maps is None:
        return _numpy_fallback(x, global_features, parents_idxs, W1, b1, W2, b2)

    from concourse.bass_utils import run_bass_kernel_spmd

    nc = _get_nc()
    res = run_bass_kernel_spmd(nc, in_maps, core_ids=list(range(NCORES)))
    _CACHE["last_result"] = res

    x = np.asarray(x, np.float32)
    out = np.empty((x.shape[0] + NPAR * NBR * BATCH, NF), np.float32)
    out[: x.shape[0]] = x
    base = x.shape[0]
    per = PPC * NBR * BATCH
    for c in range(NCORES):
        out[base + c * per : base + (c + 1) * per] = res.results[c]["out"]
    return out


# revision 12
# speedup vs baseline: 5.3934x; 1.3459x over previous
"""Trainium2 Bass kernel for nn_BranchingLayer (gnn_message_passing).

Reference computation (shapes hardcoded from the spec):
  x:[786432,32] f32, global_features:[2048,16], parents_idxs:[524288] i32,
  W1:[48,128], b1:[128], W2:[128,128], b2:[128]
  parents = x[parents_idxs]                # [524288, 32], row i = (p, b)
  h  = leaky_relu(concat(parents, g[b]) @ W1 + b1, 0.01)
  proj = h @ W2 + b2 + repeat_interleave(parents, 4, -1)
  children[(p*4+br)*2048 + b, f] = proj[p*2048+b, br*32+f]
  out = concat([x, children], 0)           # [2883584, 32]

Design (v2):
 * Shard the 256 parents over 8 cores (32/core); per-core x and output
   slices are contiguous.
 * bf16 matmuls (fp32 PE runs at 1/4 rate), fp32 PSUM accumulation.
   leaky(z) = 0.99*relu(z) + 0.01*z with the linear term folded into the
   residual matmul weights (host-precomputed in f64).  The residual
   (out += x) is kept fp32-exact by a hi/lo bf16 split of x.
 * Feature-major compute: per parent/quarter, psum1[128f,512] =
   W1'^T.xt, h1 = relu(psum1) (bf16), psum2 = W2'^T.h1 + ER^T.xt +
   E^T.xlo; DVE 32x32 block-transpose psum2 -> bt.
 * Batch columns are host-permuted: column 32c+d holds row 64d+c.  After
   the 32x32 block transpose, partition 32*br+d holds rows 64d..64d+64
   of branch br contiguously -> the output DMA is 32 descriptors x 8KB
   per branch (full line rate), one DMA per (parent, branch).
"""

import numpy as np

BATCH = 2048
NPAR = 256
NF = 32
NG = 16
NBR = 4
OFF = 262144
NCORES = 8
PPC = NPAR // NCORES          # parents per core
QW = 512                      # matmul free-dim (quarter of batch)
NQ = BATCH // QW
XROWS = 81                    # 0-31 x_hi, 32-47 g_hi, 48 ones, 49-80 x_lo

_CACHE = {}


def _build_nc(ppc=PPC, reps=1):
    import concourse.bacc as bacc
    import concourse.bass as bass
    import concourse.mybir as mybir
    import concourse.tile as tile
    from contextlib import ExitStack, nullcontext

    bf = mybir.dt.float16
    f32 = mybir.dt.float32
    nc = bacc.Bacc("TRN2", target_bir_lowering=False, debug=False)

    xt_d = nc.dram_tensor("xt", [ppc, XROWS, BATCH], bf, kind="ExternalInput")
    w1_d = nc.dram_tensor("w1", [49, 128], bf, kind="ExternalInput")
    w2_d = nc.dram_tensor("w2", [128, 128], bf, kind="ExternalInput")
    er_d = nc.dram_tensor("er", [XROWS, 128], bf, kind="ExternalInput")
    out_d = nc.dram_tensor("out", [ppc * NBR * BATCH, NF], f32, kind="ExternalOutput")

    with tile.TileContext(nc) as tc, ExitStack() as ctx:
        wpool = ctx.enter_context(tc.tile_pool(name="w", bufs=1))
        xpool = ctx.enter_context(tc.tile_pool(name="x", bufs=3))
        hpool = ctx.enter_context(tc.tile_pool(name="h", bufs=8))
        btpool = ctx.enter_context(tc.tile_pool(name="bt", bufs=3))
        p1pool = ctx.enter_context(
            tc.tile_pool(name="p1", bufs=3, space=bass.MemorySpace.PSUM)
        )
        p2pool = ctx.enter_context(
            tc.tile_pool(name="p2", bufs=3, space=bass.MemorySpace.PSUM)
        )

        w1_t = wpool.tile([49, 128], bf, tag="w1")
        nc.sync.dma_start(w1_t[:], w1_d[:])
        w2_t = wpool.tile([128, 128], bf, tag="w2")
        nc.sync.dma_start(w2_t[:], w2_d[:])
        er_t = wpool.tile([XROWS, 128], bf, tag="er")
        nc.sync.dma_start(er_t[:], er_d[:])

        rep_ctx = tc.For_i(0, reps, 1) if reps > 1 else nullcontext()
        with rep_ctx:
            for p in range(ppc):
                xt_t = xpool.tile([XROWS, BATCH], bf, tag="xt")
                nc.sync.dma_start(xt_t[:], xt_d[p])
                bt_t = btpool.tile([128, BATCH], f32, tag="bt")
                for q in range(NQ):
                    s = slice(q * QW, (q + 1) * QW)
                    ps1 = p1pool.tile([128, QW], f32, tag="ps1")
                    nc.tensor.matmul(
                        ps1[:], w1_t[:], xt_t[:49, s], start=True, stop=True
                    )
                    h1 = hpool.tile([128, QW], bf, tag="h1")
                    nc.scalar.activation(
                        h1[:], ps1[:], mybir.ActivationFunctionType.Relu
                    )
                    ps2 = p2pool.tile([128, QW], f32, tag="ps2")
                    nc.tensor.matmul(ps2[:], w2_t[:], h1[:], start=True, stop=False)
                    nc.tensor.matmul(
                        ps2[:], er_t[:], xt_t[:, s], start=False, stop=True
                    )
                    nc.vector.transpose(bt_t[:, s], ps2[:])
                for br in range(NBR):
                    row0 = (p * NBR + br) * BATCH
                    dst = out_d[row0 : row0 + BATCH, :].rearrange(
                        "(d c) f -> d (c f)", d=32
                    )
                    src = bt_t[32 * br : 32 * (br + 1), :]
                    nc.gpsimd.dma_start(dst, src)
    nc.compile()
    return nc


def _get_nc():
    if "nc" not in _CACHE:
        _CACHE["nc"] = _build_nc()
    return _CACHE["nc"]


def _perm_cols(a):
    """Permute the trailing batch axis: position 32c+d <- row 64d+c."""
    shp = a.shape[:-1]
    return np.ascontiguousarray(
        a.reshape(*shp, 32, 64).swapaxes(-1, -2).reshape(*shp, BATCH)
    )


def _pack_inputs(x, global_features, parents_idxs, W1, b1, W2, b2, ppc=PPC):
    """Build the per-core input maps (host-side sharding + layout)."""
    bf16 = np.float16
    x = np.asarray(x, np.float32)
    g = np.asarray(global_features, np.float32)
    idx = np.asarray(parents_idxs)
    W1 = np.asarray(W1, np.float32)
    b1 = np.asarray(b1, np.float32)
    W2 = np.asarray(W2, np.float32)
    b2 = np.asarray(b2, np.float32)

    n_rows = NPAR * BATCH
    exp = np.arange(n_rows, dtype=np.int64)
    if np.array_equal(idx, exp + OFF):
        parents = x[OFF : OFF + n_rows]
    else:
        parents = x[idx]  # general gather
    gi = idx.astype(np.int64) % BATCH
    if not np.array_equal(gi, np.tile(np.arange(BATCH, dtype=np.int64), NPAR)):
        return None

    # Feature-major per-parent x with permuted batch columns
    xf = parents.reshape(NPAR, BATCH, NF).transpose(0, 2, 1)  # [P, 32, B]
    xf = _perm_cols(xf)
    x_hi = xf.astype(bf16)
    x_lo = (xf - x_hi.astype(np.float32)).astype(bf16)
    g_hi = _perm_cols(np.ascontiguousarray(g.T)).astype(bf16)  # [16, B]

    xt = np.empty((NPAR, XROWS, BATCH), bf16)
    xt[:, :32] = x_hi
    xt[:, 32:48] = g_hi[None]
    xt[:, 48] = np.float32(1.0)
    xt[:, 49:81] = x_lo

    W1f = W1.astype(np.float64)
    W2f = W2.astype(np.float64)
    lin = 0.01 * (W1f @ W2f)  # [48, 128]
    w1 = np.concatenate([W1, b1[None]], axis=0).astype(bf16)  # [49, 128]
    w2 = (0.99 * W2f).astype(bf16)
    er = np.zeros((XROWS, 128), np.float64)
    jj = np.arange(128)
    er[jj // 4, jj] = 1.0
    er[:48] += lin
    er[48] = b2.astype(np.float64) + 0.01 * (b1.astype(np.float64) @ W2f)
    er[49 + jj // 4, jj] = 1.0
    er = er.astype(bf16)

    ncores = NPAR // ppc
    in_maps = []
    for c in range(ncores):
        in_maps.append(
            {
                "xt": xt[c * ppc : (c + 1) * ppc],
                "w1": w1,
                "w2": w2,
                "er": er,
            }
        )
    return in_maps


def _numpy_fallback(x, global_features, parents_idxs, W1, b1, W2, b2):
    x = np.asarray(x, np.float32)
    g = np.asarray(global_features, np.float32)
    idx = np.asarray(parents_idxs).astype(np.int64)
    pf = x[idx]
    pg = g[idx % BATCH]
    h = np.concatenate([pf, pg], axis=-1) @ np.asarray(W1, np.float32) + b1
    h = np.where(h > 0, h, 0.01 * h).astype(np.float32)
    proj = h @ np.asarray(W2, np.float32) + b2
    proj = proj + np.repeat(pf, NBR, axis=-1)
    m = proj.reshape(NPAR, BATCH, NF * NBR)
    m = np.swapaxes(m, 1, 2)
    m = m.reshape(NPAR * NBR, NF, BATCH)
    m = np.swapaxes(m, 1, 2)
    children = m.reshape(NPAR * NBR * BATCH, NF)
    return np.concatenate([x, children], axis=0).astype(np.float32)


def kernel(x, global_features, parents_idxs, W1, b1, W2, b2):
    in_maps = _pack_inputs(x, global_features, parents_idxs, W1, b1, W2, b2)
    if in_maps is None:
        return _numpy_fallback(x, global_features, parents_idxs, W1, b1, W2, b2)

    from concourse.bass_utils import run_bass_kernel_spmd

    nc = _get_nc()
    res = run_bass_kernel_spmd(nc, in_maps, core_ids=list(range(NCORES)))
    _CACHE["last_result"] = res

    x = np.asarray(x, np.float32)
    out = np.empty((x.shape[0] + NPAR * NBR * BATCH, NF), np.float32)
    out[: x.shape[0]] = x
    base = x.shape[0]
    per = PPC * NBR * BATCH
    for c in range(NCORES):
        out[base + c * per : base + (c + 1) * per] = res.results[c]["out"]
    return out
